# revision 1
# baseline (speedup 1.0000x reference)
"""Trainium2 Bass kernel for the 1-D Bessel (von Mises-like) kernel matrix:

    K[i, j] = I0(2a * cos(pi * (x_i - y_j))) * exp(-2a),   a = 10

Algorithm
---------
K depends on d = x_i - y_j only through the periodic even function
h(d) = I0(20 cos(pi d)) e^-20, which has period 1.  Its log has a rapidly
converging Fourier cosine series:

    log h(d) = b0 + sum_{k=1..63} b_k cos(2 pi k d)          (|err| < 3e-8)

and cos(2 pi k (x - y)) = cos(2pi k x) cos(2pi k y) + sin(2pi k x) sin(2pi k y),
so log K is a rank-127 product of small trig feature matrices:

    log K = U.T @ V,   U, V in R^[128 x n]  (row 127 zero-padded)

On each NeuronCore (rows of x sharded 8 ways, y replicated) the rank-128
contraction runs on the TensorEngine as TWO bf16 matmul passes accumulated
in fp32 PSUM:

  pass 1:  Uh.T @ Vh            (bf16 hi parts of all 128 feature rows)
  pass 2:  Uc.T @ Vc            (packed cross corrections: for the constant
           + top-31 harmonics, rows [Uh_s | Ul_s] x [Vl_s | Vh_s], K=126,
           capturing uh*vl + ul*vh; tail harmonics are < 3e-5 and need no
           correction)

giving ~1.3e-4 max relative error.  A fused exp() on the Scalar engine
moves PSUM->SBUF, emitting K * 2^16 in float16 (K spans [2.1e-9, 0.09] =
7.6 decades; fp16 normals span 9 decades, so the scaled value always
stays in the normal range and costs only a 4.9e-4 half-ulp rounding).
The host multiplies by the exact 2^-16 and upcasts.  Halving the output
bytes makes the kernel compute-bound at the Scalar engine's exp pass
(~64 us) with the 16 MiB/core output DMA (~47 us) hidden under it.

The tiny [128 x 8192] trig features are precomputed on host in float64.
"""

import os
import sys

import numpy as np

sys.path.insert(0, "/opt/trn_rl_repo")

A = 10.0
NX = 8192
NY = 8192
N_CORES = 8
MX = NX // N_CORES  # 1024 rows of x per core
KH = 63   # harmonics; rank = 1 + 2*63 = 127 (+1 zero pad = 128)
KS = 31   # harmonics getting hi/lo cross correction (+ constant row)

# Fourier cosine coefficients of log(I0(20 cos(pi d))) - 20 on d in [0, 1),
# computed offline in float64 via FFT of the exact series evaluation.
_B0 = -9.320623105523872
_BK = [
    7.970447139028089, -1.4358756600553582, 0.5530401566383198,
    -0.27432647869384885, 0.1547723650507224, -0.09433791302730635,
    0.060502068515108406, -0.04020530135648252, 0.027418113277826187,
    -0.01906554834357182, 0.013458315954332174, -0.009613552975863679,
    0.0069329638057468446, -0.005038947804517573, 0.003686131354141929,
    -0.00271122806102214, 0.00200343687917714, -0.0014863506699641636,
    0.00110656955440988, -0.0008263523699001975, 0.000618771677773785,
    -0.00046446052148687905, 0.00034939361165105417, -0.0002633536495551932,
    0.00019885898700602698, -0.0001504063999160173, 0.00011393178617259052,
    -8.642320754869491e-05, 6.564143485541695e-05, -4.991697831321222e-05,
    3.8001927162546077e-05, -2.8961314711295418e-05, 2.209314682322636e-05,
    -1.686932038817502e-05, 1.2891834155415738e-05, -9.86023888809833e-06,
    7.54737769766621e-06, -5.781261162339443e-06, 4.431495660336892e-06,
    -3.399100216289112e-06, 2.6088513344058884e-06, -2.0035181213087346e-06,
    1.5395138373841213e-06, -1.1836108673737676e-06, 9.104555226369233e-07,
    -7.006854327413115e-07, 5.395016369359441e-07, -4.1558428389927703e-07,
    3.202683473607116e-07, -2.469163527350026e-07, 1.9044056002308284e-07,
    -1.469386541959237e-07, 1.1341573524768808e-07, -8.757198758072422e-08,
    6.764038400573971e-08, -5.2262540395907754e-08, 4.039368538745272e-08,
    -3.122986684565119e-08, 2.4152156136794418e-08, -1.868385388963757e-08,
    1.4457648827642462e-08, -1.1190400014929511e-08, 8.663762585260409e-09,
]

_NC_CACHE = None
LAST_EXEC_TIME_NS = None
LAST_TRACE_PATH = None


def _features(x, y):
    """Host-side float64 trig features -> bf16 matmul operands.

    Feature layout (128 rows): row 0 = constant, rows 1..63 = cos harmonics,
    rows 64..126 = sin harmonics, row 127 = 0.  Coefficients b_k are folded
    into the U (x) side.

    Returns (uxh, uxc, vyh, vyc), all bf16:
      uxh/vyh [128, n]: bf16 hi parts of U / V.
      uxc/vyc [128, n]: packed correction operands over the split set
        (constant + cos/sin harmonics 1..KS, 63 rows):
        uxc = [Uh_s ; bf16(U_s - Uh_s)],  vyc = [bf16(V_s - Vh_s) ; Vh_s],
        so uxc.T @ vyc accumulates uh*vl + ul*vh for the split rows.
    """
    import ml_dtypes

    bf16 = ml_dtypes.bfloat16

    xf = np.asarray(x, np.float32).reshape(-1).astype(np.float64)
    yf = np.asarray(y, np.float32).reshape(-1).astype(np.float64)
    ks = np.arange(1, KH + 1, dtype=np.float64)[:, None]
    bk = np.array(_BK, np.float64)[:, None]

    ang_x = (2.0 * np.pi) * ks * xf[None, :]
    u = np.empty((128, xf.size), np.float32)
    u[0] = _B0 + 16.0 * 0.6931471805599453  # fold the 2^16 fp16 scale in
    u[1 : KH + 1] = bk * np.cos(ang_x)
    u[KH + 1 : 2 * KH + 1] = bk * np.sin(ang_x)
    u[127] = 0.0

    ang_y = (2.0 * np.pi) * ks * yf[None, :]
    v = np.empty((128, yf.size), np.float32)
    v[0] = 1.0
    v[1 : KH + 1] = np.cos(ang_y)
    v[KH + 1 : 2 * KH + 1] = np.sin(ang_y)
    v[127] = 0.0

    uh = u.astype(bf16)
    vh = v.astype(bf16)
    ul = (u - uh.astype(np.float32)).astype(bf16)
    vl = (v - vh.astype(np.float32)).astype(bf16)

    # split set: constant + cos 1..KS + sin 1..KS  (63 rows)
    split = np.r_[0, np.arange(1, KS + 1), np.arange(KH + 1, KH + 1 + KS)]
    ns = split.size  # 63
    uc = np.zeros((128, xf.size), bf16)
    vc = np.zeros((128, yf.size), bf16)
    uc[:ns] = uh[split]
    uc[ns : 2 * ns] = ul[split]
    vc[:ns] = vl[split]
    vc[ns : 2 * ns] = vh[split]
    return uh, uc, vh, vc


def _build():
    """Build + compile the per-core Bass/Tile kernel (cached)."""
    global _NC_CACHE
    if _NC_CACHE is not None:
        return _NC_CACHE

    from concourse import bacc, mybir
    import concourse.tile as tile

    f32 = mybir.dt.float32
    f16 = mybir.dt.float16
    bf16 = mybir.dt.bfloat16

    nc = bacc.Bacc(
        "TRN2", target_bir_lowering=False, debug=False, num_devices=N_CORES
    )
    ux_d = nc.dram_tensor("ux", [128, 2 * MX], bf16, kind="ExternalInput").ap()
    vy_d = nc.dram_tensor("vy", [128, 2 * NY], bf16, kind="ExternalInput").ap()
    out_d = nc.dram_tensor("out", [MX, NY], f16, kind="ExternalOutput").ap()

    n_mt = MX // 128   # 8 row blocks
    n_ng = NY // 2048  # 4 psum-sized col groups of 2048

    with tile.TileContext(nc) as tc:
        with (
            tc.tile_pool(name="wpool", bufs=1) as wpool,
            tc.tile_pool(name="vpool", bufs=2 * n_ng) as vpool,
            tc.tile_pool(name="pspool", bufs=2, space="PSUM") as pspool,
            tc.tile_pool(name="opool", bufs=3) as opool,
        ):
            # input loads, few large DMAs (each DMA issue costs ~0.65 us on
            # the sync sequencer, so issue count dominates the head):
            # ux = [uxh | uxc] in one tile, vy = per-group [vh | vc] tiles,
            # with group 0 split in two so the first matmuls start early
            ux_t = wpool.tile([128, 2 * MX], bf16, name="ux_t", tag="ux_t")
            vys = []
            for ng in range(n_ng):
                vy_t = vpool.tile([128, 4096], bf16, name=f"vy_{ng}", tag="vy")
                vys.append(vy_t)
            nc.sync.dma_start(ux_t[:], ux_d[:])
            v0d = vys[0].rearrange("p (two c) -> p two c", two=2)
            s0d = vy_d[:, 0:4096].rearrange("p (two c) -> p two c", two=2)
            nc.sync.dma_start(v0d[:, :, 0:1024], s0d[:, :, 0:1024])
            nc.sync.dma_start(v0d[:, :, 1024:2048], s0d[:, :, 1024:2048])

            # PE warm-up: dummy matmuls on a zeroed tile keep the PE busy
            # while inputs stream in, so the HAM clock gate is at 2.4 GHz
            # when the real matmuls start (first group runs 2x faster).
            # The <=2 us gap before the real stream is shorter than the
            # 3.4 us HAM idle window, so the clock stays warm.
            warm_t = wpool.tile([128, 512], bf16, name="warm_t", tag="warm_t")
            nc.vector.memset(warm_t[:], 0.0)
            warm_ps = pspool.tile([128, 512], f32, name="warm_ps", tag="ps")
            for _w in range(12):
                nc.tensor.matmul(
                    warm_ps[:, 0:512],
                    warm_t[:, 0:128],
                    warm_t[:],
                    start=True,
                    stop=True,
                )
            for ng in range(1, n_ng):
                sl = slice(ng * 4096, (ng + 1) * 4096)
                nc.sync.dma_start(vys[ng][:], vy_d[:, sl])

            # skew the first two row blocks (m0h0, m1h0, m0h1, m1h1) so the
            # first four pieces consume only the already-resident vy0/vy1,
            # giving the input queue 4 extra us to land vy2/vy3 — targets
            # the 2.7 us of deterministic early ACT-window gaps
            order = [(0, 0), (1, 0), (0, 1), (1, 1)] + [
                (mm, hh) for mm in range(2, n_mt) for hh in range(2)
            ]
            for m, half in order:
                msl = slice(m * 128, (m + 1) * 128)
                if True:
                    out_t = opool.tile(
                        [128, NY // 2], f16, name=f"out_{m}_{half}", tag="out_t"
                    )
                    for sub in range(n_ng // 2):
                        ng = half * (n_ng // 2) + sub
                        ps = pspool.tile(
                            [128, 2048], f32, name=f"ps_{m}_{ng}", tag="ps"
                        )
                        for s in range(4):
                            ssl = slice(s * 512, (s + 1) * 512)
                            nc.tensor.matmul(
                                ps[:, ssl],
                                ux_t[:, msl],
                                vys[ng][:, s * 512 : (s + 1) * 512],
                                start=True, stop=False,
                            )
                            nc.tensor.matmul(
                                ps[:, ssl],
                                ux_t[:, MX + m * 128 : MX + (m + 1) * 128],
                                vys[ng][:, 2048 + s * 512 : 2048 + (s + 1) * 512],
                                start=False, stop=True,
                            )
                        # fp16 out = exp(L + 16 ln2) = K * 2^16 (the scale is
                        # folded into the constant feature row), always in
                        # fp16 normal range; host rescales by exact 2^-16
                        nc.scalar.activation(
                            out_t[:, sub * 2048 : (sub + 1) * 2048],
                            ps[:],
                            mybir.ActivationFunctionType.Exp,
                        )
                    if m == n_mt - 1:
                        # last row block: store per 2048-col group right
                        # behind each exp so the queue drains with the ACT
                        # stream and the kernel tail stays short
                        for q in range(2):
                            cols = half * (NY // 2) + q * 2048
                            nc.sync.dma_start(
                                out_d[msl, cols : cols + 2048],
                                out_t[:, q * 2048 : (q + 1) * 2048],
                            )
                    else:
                        nc.sync.dma_start(
                            out_d[msl, half * (NY // 2) : (half + 1) * (NY // 2)],
                            out_t[:],
                        )

    nc.compile()
    _NC_CACHE = nc
    return nc


def kernel(x: np.ndarray, y: np.ndarray) -> np.ndarray:
    global LAST_EXEC_TIME_NS, LAST_TRACE_PATH
    from concourse import bass_utils

    uh, uc, vh, vc = _features(x, y)
    nc = _build()

    # vy blocks: [vh_ng | vc_ng] per 2048-column group
    vy = np.concatenate(
        [
            np.concatenate(
                [vh[:, g * 2048 : (g + 1) * 2048], vc[:, g * 2048 : (g + 1) * 2048]],
                axis=1,
            )
            for g in range(NY // 2048)
        ],
        axis=1,
    )
    in_maps = [
        {
            "ux": np.concatenate(
                [uh[:, i * MX : (i + 1) * MX], uc[:, i * MX : (i + 1) * MX]],
                axis=1,
            ),
            "vy": vy,
        }
        for i in range(N_CORES)
    ]
    trace = bool(os.environ.get("BESSEL_TRACE"))
    res = bass_utils.run_bass_kernel_spmd(
        nc, in_maps, core_ids=list(range(N_CORES)), trace=trace
    )
    LAST_EXEC_TIME_NS = res.exec_time_ns
    if res.instructions_and_trace is not None:
        LAST_TRACE_PATH = res.instructions_and_trace[1]
    out = np.empty((NX, NY), np.float32)
    for i in range(N_CORES):
        blk = out[i * MX : (i + 1) * MX]
        np.multiply(
            res.results[i]["out"].astype(np.float32),
            np.float32(2.0**-16),
            out=blk,
        )
    return out



# revision 14
# speedup vs baseline: 1.3254x; 1.3254x over previous
"""Trainium2 Bass kernel for the 1-D Bessel (von Mises-like) kernel matrix:

    K[i, j] = I0(2a * cos(pi * (x_i - y_j))) * exp(-2a),   a = 10

Algorithm
---------
K depends on d = x_i - y_j only through the periodic even function
h(d) = I0(20 cos(pi d)) e^-20.  Unlike log h (which needs 63 harmonics),
h ITSELF has a classical cosine expansion with super-exponentially
decaying coefficients (I0(2a cos t) = sum_k I_k(a)^2 e^{2ikt}):

    h(d) = c0 + sum_{k=1..16} c_k cos(2 pi k d),  c_k = 2 e^-2a I_k(a)^2
    (|c_17 tail| < 1e-12)

so K is directly a rank-33 trig outer product -- NO exp on device at all:

    K = U.T @ V,  U, V in R^[33 x n]

On each NeuronCore (rows of x sharded 8 ways, y replicated) this runs as
ONE bf16 matmul pass (PE cycles depend only on output columns, not the
contraction rank, so hi/lo bf16 cross-corrections for the constant +
top-7 harmonics ride along for free in rows 33..62, rank 63 <= 128):

    rows  0..32 : Uh        . Vh      (bf16 hi of all features)
    rows 33..47 : (U-Uh)_s  . Vh_s    (lo x hi, split set: const + k<=7)
    rows 48..62 : Uh_s      . (V-Vh)_s

The fp32 PSUM result (K * 2^16, the scale folded into U's coefficients so
all outputs sit in the fp16 normal range) is cast PSUM->SBUF to fp16 by
the Scalar AND Vector engines working in parallel (the baseline's 64 us
Scalar-only exp pass is gone entirely), then streamed to HBM as fp16.
The kernel is bound by the 16 MiB/core output DMA (~47 us at the 358
GB/s per-core HBM limit); matmul (27 us) and the split casts (35 us)
hide under it.  The host multiplies by the exact 2^-16 and upcasts.
Total error ~2.1e-4 L2, dominated by fp16 output rounding.
"""

import os
import sys

import numpy as np

sys.path.insert(0, "/opt/trn_rl_repo")

A = 10.0
NX = 8192
NY = 8192
N_CORES = 8
MX = NX // N_CORES  # 1024 rows of x per core
KH = 16   # harmonics of h: base rank = 1 + 2*16 = 33
KS = 7    # harmonics getting two-sided hi/lo correction (+ constant row)
NROWS = 64  # 33 base + 2*(1+2*KS)=30 correction rows + 1 zero row shipped
# The matmul still contracts over K=128: rows 67..127 of the lhsT are zero
# and rows 67..127 of the rhs are zeroed on-chip (GpSimd memset).  K=128
# keeps all four 32-row groups of the PE array active -- with K=64 the HAM
# activity monitor never un-throttles the PE clock from 1.2 to 2.4 GHz
# (measured: 427ns vs 216ns per 512-col matmul), and PE cycles depend only
# on output columns, not K.  Shipping just 67 vy rows (1.05 MB instead of
# 2 MB) trims the input share of the DMA-engine critical path.
RANK = 128
LOG2_SCALE = 16  # fold 2^16 into coefficients: outputs in fp16 normal range

_NC_CACHE = None
LAST_EXEC_TIME_NS = None
LAST_TRACE_PATH = None


def _coeffs():
    """Cosine-series coefficients of h(d) = I0(20 cos(pi d)) e^-20 on [0,1),
    computed in float64 via FFT of an exact dense sampling (aliasing error
    ~c_8175, i.e. zero)."""
    n = 8192
    d = np.arange(n) / n
    h = np.i0(2.0 * A * np.cos(np.pi * d)) * np.exp(-2.0 * A)
    c = np.real(np.fft.rfft(h)) / n
    c[1:] *= 2.0
    return c[: KH + 1]  # c_0 .. c_16


def _features(x, y):
    """Host-side float64 trig features -> packed bf16 matmul operands.

    Returns (U [128, nx], V [64, ny]), both bf16:
      rows 0..32  : hi parts   (row 0 const, 1..16 cos, 17..32 sin; the
                    c_k * 2^16 coefficients folded into the U side)
      rows 33..47 : U: lo parts of split set; V: hi parts of split set
      rows 48..62 : U: hi parts of split set; V: lo parts of split set
      row 63 zero; U rows 64..127 zero (matching V rows zeroed on-chip)
    where the split set = const + cos 1..KS + sin 1..KS (15 rows), so
    U.T @ V = uh.vh + ul_s.vh_s + uh_s.vl_s (full bf16-pair precision on
    the dominant coefficients; the rest are < 6e-5 of the total).
    """
    import ml_dtypes

    bf16 = ml_dtypes.bfloat16
    ck = _coeffs() * float(2.0**LOG2_SCALE)

    xf = np.asarray(x, np.float32).reshape(-1).astype(np.float64)
    yf = np.asarray(y, np.float32).reshape(-1).astype(np.float64)
    ks = np.arange(1, KH + 1, dtype=np.float64)[:, None]

    ang_x = (2.0 * np.pi) * ks * xf[None, :]
    u = np.empty((2 * KH + 1, xf.size), np.float64)
    u[0] = ck[0]
    u[1 : KH + 1] = ck[1:, None] * np.cos(ang_x)
    u[KH + 1 :] = ck[1:, None] * np.sin(ang_x)

    ang_y = (2.0 * np.pi) * ks * yf[None, :]
    v = np.empty((2 * KH + 1, yf.size), np.float64)
    v[0] = 1.0
    v[1 : KH + 1] = np.cos(ang_y)
    v[KH + 1 :] = np.sin(ang_y)

    uh = u.astype(bf16)
    vh = v.astype(bf16)
    ul = (u - uh.astype(np.float64)).astype(bf16)
    vl = (v - vh.astype(np.float64)).astype(bf16)

    split = np.r_[0, np.arange(1, KS + 1), np.arange(KH + 1, KH + 1 + KS)]
    ns = split.size  # 15
    nb = 2 * KH + 1  # 33

    U = np.zeros((RANK, xf.size), bf16)
    V = np.zeros((NROWS, yf.size), bf16)
    U[:nb] = uh
    V[:nb] = vh
    U[nb : nb + ns] = ul[split]
    V[nb : nb + ns] = vh[split]
    U[nb + ns : nb + 2 * ns] = uh[split]
    V[nb + ns : nb + 2 * ns] = vl[split]
    return U, V


def _cast_schedule(n_tiles):
    """Greedy balance of PSUM->SBUF cast tiles between ACT (0.996us) and
    DVE (1.19us): returns list of 'act'/'dve', both finishing ~equal."""
    t_act, t_dve = 0.0, 0.0
    out = []
    for _ in range(n_tiles):
        if t_act + 0.996 <= t_dve + 1.19:
            out.append("act")
            t_act += 0.996
        else:
            out.append("dve")
            t_dve += 1.19
    return out


def _build():
    """Build + compile the per-core Bass/Tile kernel (cached)."""
    global _NC_CACHE
    if _NC_CACHE is not None:
        return _NC_CACHE

    from concourse import bacc, mybir
    import concourse.tile as tile

    f32 = mybir.dt.float32
    f16 = mybir.dt.float16
    bf16 = mybir.dt.bfloat16

    nc = bacc.Bacc(
        "TRN2", target_bir_lowering=False, debug=False, num_devices=N_CORES
    )
    ux_d = nc.dram_tensor("ux", [NROWS, MX], bf16, kind="ExternalInput").ap()
    vy_d = nc.dram_tensor("vy", [NROWS, NY], bf16, kind="ExternalInput").ap()
    out_d = nc.dram_tensor("out", [MX, NY], f16, kind="ExternalOutput").ap()

    n_mt = MX // 128       # 8 row blocks
    sched = _cast_schedule(n_mt * 8)

    # process half-rows (m, half) so (a) the first two row blocks consume
    # the vy chunks at half pace (no stall while inputs stream in) and
    # (b) output DMAs fire at 1 MiB granularity, keeping the DMA engines
    # fed instead of bursting once per 2 MiB row
    order = [(0, 0), (1, 0), (0, 1), (1, 1)] + [
        (m, h) for m in range(2, n_mt) for h in range(2)
    ]

    with tile.TileContext(nc) as tc:
        with (
            tc.tile_pool(name="wpool", bufs=1) as wpool,
            tc.tile_pool(name="pspool", bufs=4, space="PSUM") as pspool,
            tc.tile_pool(name="opool", bufs=8) as opool,
        ):
            # input loads on the Scalar HWDGE queue (the Sync queue carries
            # only output stores, so input issues never FIFO-block them):
            # ux (256 KB) + vy rows 0..63 in pipelined 2048-col chunks
            # (256 KB each); rows 64..127 of each vy chunk are zeroed by
            # the otherwise-idle GpSimd engine (1.7us per chunk) instead of
            # shipping zeros through the DMA engines
            ux_t = wpool.tile([RANK, MX], bf16, name="ux_t", tag="ux_t")
            vy_t = wpool.tile([RANK, NY], bf16, name="vy_t", tag="vy_t")
            nc.scalar.dma_start(ux_t[0:NROWS, :], ux_d[:])
            nc.gpsimd.memset(ux_t[NROWS:RANK, :], 0.0)
            nc.scalar.dma_start(vy_t[0:NROWS, 0:1024], vy_d[:, 0:1024])
            nc.gpsimd.memset(vy_t[NROWS:RANK, 0:1024], 0.0)
            nc.scalar.dma_start(vy_t[0:NROWS, 1024:2048], vy_d[:, 1024:2048])
            nc.gpsimd.memset(vy_t[NROWS:RANK, 1024:2048], 0.0)

            # PE warm-up: dummy matmuls on a zeroed tile keep the PE busy
            # while inputs stream in, so the HAM clock gate is at 2.4 GHz
            # when the real matmuls start.
            warm_t = wpool.tile([128, 512], bf16, name="warm_t", tag="warm_t")
            nc.vector.memset(warm_t[:], 0.0)
            warm_ps = pspool.tile([128, 512], f32, name="warm_ps", tag="ps")
            for _w in range(7):
                nc.tensor.matmul(
                    warm_ps[:, 0:512],
                    warm_t[:, 0:128],
                    warm_t[:],
                    start=True,
                    stop=True,
                )
            for chunk in range(1, 4):
                sl = slice(chunk * 2048, (chunk + 1) * 2048)
                nc.scalar.dma_start(vy_t[0:NROWS, sl], vy_d[:, sl])
                nc.gpsimd.memset(vy_t[NROWS:RANK, sl], 0.0)

            ti = 0
            for m, half in order:
                msl = slice(m * 128, (m + 1) * 128)
                out_t = opool.tile(
                    [128, NY // 2], f16, name=f"out_{m}_{half}", tag="out_t"
                )
                for g in range(4):
                    ps = pspool.tile(
                        [128, 1024], f32, name=f"ps_{m}_{half}_{g}", tag="ps"
                    )
                    for s in range(2):
                        col = half * 4096 + g * 1024 + s * 512
                        nc.tensor.matmul(
                            ps[:, s * 512 : (s + 1) * 512],
                            ux_t[:, msl],
                            vy_t[:, col : col + 512],
                            start=True,
                            stop=True,
                        )
                    osl = slice(g * 1024, (g + 1) * 1024)
                    if sched[ti] == "act":
                        nc.scalar.copy(out_t[:, osl], ps[:])
                    else:
                        nc.vector.tensor_copy(out_t[:, osl], ps[:])
                    ti += 1
                    # store per 2048-col chunk right behind each cast pair:
                    # the output stream starts earliest and feeds the DMA
                    # engines steadily instead of bursting per half-row
                    if g % 2 == 1:
                        cols = slice((g - 1) * 1024, (g + 1) * 1024)
                        dcols = slice(
                            half * 4096 + (g - 1) * 1024,
                            half * 4096 + (g + 1) * 1024,
                        )
                        nc.sync.dma_start(out_d[msl, dcols], out_t[:, cols])

    nc.compile()
    _NC_CACHE = nc
    return nc


def kernel(x: np.ndarray, y: np.ndarray) -> np.ndarray:
    global LAST_EXEC_TIME_NS, LAST_TRACE_PATH
    from concourse import bass_utils

    U, V = _features(x, y)
    nc = _build()

    in_maps = [
        {
            "ux": np.ascontiguousarray(U[:, i * MX : (i + 1) * MX]),
            "vy": V,
        }
        for i in range(N_CORES)
    ]
    trace = bool(os.environ.get("BESSEL_TRACE"))
    res = bass_utils.run_bass_kernel_spmd(
        nc, in_maps, core_ids=list(range(N_CORES)), trace=trace
    )
    LAST_EXEC_TIME_NS = res.exec_time_ns
    if res.instructions_and_trace is not None:
        LAST_TRACE_PATH = res.instructions_and_trace[1]
    out = np.empty((NX, NY), np.float32)
    for i in range(N_CORES):
        blk = out[i * MX : (i + 1) * MX]
        np.multiply(
            res.results[i]["out"].astype(np.float32),
            np.float32(2.0**-LOG2_SCALE),
            out=blk,
        )
    return out
